# revision 22
# baseline (speedup 1.0000x reference)
"""GQA attention kernel for 8 TRN2 NeuronCores — chunk-pipelined v2.

Problem: B=2, T=2048, C=4096, NH=32 q-heads, NKV=8 kv-heads, HD=128,
RoPE (theta=1e4), causal, f32 I/O.

Sharding: core = (batch b, head-group g): b = core//4, g = core%4.
Each core owns batch b, kv heads {2g, 2g+1} (= q heads 8g..8g+7).

v2 structure (vs phase-serial v1): token chunks of 512 are processed
in a single pipeline — project Q/K/V for chunk qc, then immediately
run causal attention of all 8 heads against K/V chunks <= qc. The
ACT-engine exp of chunk qc then overlaps the PE projections of chunk
qc+1, keeping the PE warm (HAM K=8/8) through the whole attention
phase instead of oscillating. exp is issued in [128, 2, 512] groups
to amortize the ~352-cycle ACTIVATE overhead.

RoPE rotate_half runs on the DVE as a 32-lane quadrant shuffle: q/k
head features are permuted on the host (scores are invariant under a
shared q/k permutation) so each rope pair sits 16 partitions apart
within a quadrant, and the sign folds into the sin table.
"""

import sys

sys.path.insert(0, "/opt/trn_rl_repo")

import numpy as np
import ml_dtypes

import concourse.bass as bass
import concourse.bacc as bacc
import concourse.mybir as mybir
import concourse.tile as tile
from concourse.bass_utils import run_bass_kernel_spmd

BF16 = mybir.dt.bfloat16
F32 = mybir.dt.float32
AF = mybir.ActivationFunctionType
ALU = mybir.AluOpType

B, T, C = 2, 2048, 4096
NH, NKV, HD = 32, 8, 128
THETA = 10000.0
NCORES = 8

QH = 8          # q heads per core
KV = 2          # kv heads per core
QC = 4          # token chunks of 512
CCH = 32        # contraction chunks of 128 over C
NOUT = 12      # projection out tiles per chunk: k0,k1,v0,v1,q0..q7

ROT_MASK = [(i + 16) % 32 for i in range(32)]

_CACHE = {}


def _build_nc():
    nc = bacc.Bacc("TRN2", target_bir_lowering=False, debug=False,
                   enable_asserts=False, num_devices=NCORES)

    xT_d = nc.dram_tensor("xT", [C, T], BF16, kind="ExternalInput")
    wqkv_d = nc.dram_tensor("wqkv", [NOUT, 8, 128, 512], BF16,
                            kind="ExternalInput")
    wo_d = nc.dram_tensor("wo", [128, QH, C], BF16, kind="ExternalInput")
    cos_d = nc.dram_tensor("cosT", [128, T], BF16, kind="ExternalInput")
    sin_d = nc.dram_tensor("sinT", [128, T], BF16, kind="ExternalInput")
    ident_d = nc.dram_tensor("ident", [128, 128], BF16, kind="ExternalInput")
    cmask_d = nc.dram_tensor("cmask", [128, 4, 512], F32, kind="ExternalInput")
    out_d = nc.dram_tensor("out", [T, C], F32, kind="ExternalOutput")

    with tile.TileContext(nc) as tc:
        with tc.tile_pool(name="persist", bufs=1) as pp:
            ident = pp.tile([128, 128], BF16)
            nc.sync.dma_start(ident, ident_d.ap())
            cosT = pp.tile([128, T], BF16)
            sinT = pp.tile([128, T], BF16)
            cmask = pp.tile([128, 4, 512], F32)
            nc.gpsimd.dma_start(cosT, cos_d.ap())
            nc.gpsimd.dma_start(sinT, sin_d.ap())
            nc.gpsimd.dma_start(cmask, cmask_d.ap())

            KTt = pp.tile([128, KV, T], BF16)
            QT = pp.tile([128, 2, QH, 512], BF16)   # double-buffered chunks
            OT = pp.tile([128, QH, T], BF16)
            Vn = pp.tile([128, KV, 16, 132], BF16)
            nc.vector.memset(Vn[:, :, :, 128:129], 1.0)

            X_PIECES0 = (2, 2, 4, 8, 8, 8)
            X_PIECES = (8, 8, 8, 8)

            xview = xT_d.ap().rearrange("(c p) t -> p c t", p=128)

            with tc.tile_pool(name="xp", bufs=2) as xp, \
                 tc.tile_pool(name="wtp", bufs=16) as wtp, \
                 tc.tile_pool(name="vtp", bufs=2) as vtp, \
                 tc.tile_pool(name="rtmp", bufs=2) as rtp, \
                 tc.tile_pool(name="rsp", bufs=2) as rsp, \
                 tc.tile_pool(name="ptp", bufs=10) as ptp, \
                 tc.tile_pool(name="rcp", bufs=4) as rcp, \
                 tc.tile_pool(name="pproj", bufs=2, space="PSUM") as pj, \
                 tc.tile_pool(name="pst", bufs=2, space="PSUM") as stp, \
                 tc.tile_pool(name="ppo", bufs=2, space="PSUM") as pop:

                # HAM warm-up: PE busy while first x/w DMAs land
                for w in range(40):
                    wps = pj.tile([128, 128], BF16, name=f"warm{w}", tag="pj")
                    nc.tensor.transpose(wps, ident, ident)

                def load_x(qc, pieces):
                    tsl = slice(qc * 512, (qc + 1) * 512)
                    xt = xp.tile([128, CCH, 512], BF16, name=f"xt{qc}",
                                 tag="xt")
                    c0 = 0
                    for w in pieces:
                        nc.scalar.dma_start(xt[:, c0:c0 + w, :],
                                            xview[:, c0:c0 + w, tsl])
                        c0 += w
                    return xt

                xts = [None] * QC
                xts[0] = load_x(0, X_PIECES0)

                # init st slots so stale-region exp stays finite
                for i in range(2):
                    sti = stp.tile([128, 2, 512], F32, name=f"sti{i}",
                                   tag="st")
                    nc.vector.memset(sti, 0.0)

                def rope_write(dst, ps, tsl):
                    # dst = ps*cos + quadshuffle(ps)*sin  (sign baked in sin)
                    tmp = rtp.tile([128, 512], F32)
                    nc.vector.stream_shuffle(tmp, ps, ROT_MASK)
                    rs = rsp.tile([128, 512], BF16)
                    nc.vector.tensor_tensor(rs, tmp, sinT[:, tsl],
                                            op=ALU.mult)
                    nc.vector.tensor_tensor(dst, ps, cosT[:, tsl],
                                            op=ALU.mult)
                    nc.vector.tensor_tensor(dst, dst, rs, op=ALU.add)

                for qc in range(QC):
                    tsl = slice(qc * 512, (qc + 1) * 512)
                    qcb = qc % 2

                    # prefetch next chunk's x ahead of this chunk's exps
                    # in the ACT HWDGE queue
                    if qc + 1 < QC:
                        xts[qc + 1] = load_x(qc + 1, X_PIECES)
                    xt = xts[qc]

                    # ---- projections for chunk qc ----
                    for o in range(NOUT):
                        ps = pj.tile([128, 512], F32, name=f"pj{qc}_{o}",
                                     tag="pj")
                        for cc in range(8):
                            wt = wtp.tile([128, 512], BF16)
                            nc.sync.dma_start(wt, wqkv_d.ap()[o, cc])
                            for k in range(4):
                                c = cc * 4 + k
                                nc.tensor.matmul(
                                    ps, wt[:, k * 128:(k + 1) * 128],
                                    xt[:, c, :],
                                    start=(c == 0), stop=(c == CCH - 1))
                        if o < 2:
                            rope_write(KTt[:, o, tsl], ps, tsl)
                        elif o < 10:
                            h = o - 2
                            rope_write(QT[:, qcb, h, :], ps, tsl)
                        else:
                            kvi = o - 10
                            vt = vtp.tile([128, 512], BF16)
                            nc.vector.tensor_copy(vt, ps)
                            # blocked XBAR transpose: [hd,512] -> 4x[tok,hd]
                            # (contiguous staging: strided transpose targets
                            # are silently mis-written by the XBAR path)
                            vc = vtp.tile([128, 4, 128], BF16, tag="vc")
                            nc.scalar.dma_start_transpose(vc, vt)
                            nc.vector.tensor_copy(
                                Vn[:, kvi, qc * 4:(qc + 1) * 4, 0:128], vc)

                    # ---- attention for chunk qc, all heads ----
                    NG = 2 * qc + 2
                    for h in range(QH):
                        kv = h // 4
                        pts = []
                        for g in range(NG):
                            st = stp.tile([128, 2, 512], F32, tag="st")
                            for i in range(2):
                                kt = 2 * g + i
                                d = kt - 4 * qc
                                ksl = slice(kt * 128, (kt + 1) * 128)
                                if d < 0:
                                    nc.tensor.matmul(
                                        st[:, i, :], KTt[:, kv, ksl],
                                        QT[:, qcb, h, :],
                                        start=True, stop=True)
                                else:
                                    nc.tensor.matmul(
                                        st[:, i, d * 128:],
                                        KTt[:, kv, ksl],
                                        QT[:, qcb, h, d * 128:],
                                        start=True, stop=True)
                            if g >= NG - 2:
                                # only the diagonal-straddling 128-block
                                # needs masking; the below-block region is
                                # stale psum that AV never reads
                                for i in range(2):
                                    d = 2 * g + i - 4 * qc
                                    bsl = slice(d * 128, (d + 1) * 128)
                                    nc.vector.tensor_tensor(
                                        st[:, i, bsl], st[:, i, bsl],
                                        cmask[:, d, bsl], op=ALU.add)
                            ptile = ptp.tile([128, 2, 512], BF16)
                            nc.scalar.activation(ptile, st, AF.Exp)
                            pts.append(ptile)
                        for j in range(4):
                            qt = 4 * qc + j
                            po = pop.tile([128, 129], F32, tag="po")
                            for kt in range(qt + 1):
                                nc.tensor.matmul(
                                    po,
                                    pts[kt // 2][:, kt % 2,
                                                 j * 128:(j + 1) * 128],
                                    Vn[:, kv, kt, 0:129],
                                    start=(kt == 0), stop=(kt == qt))
                            rc = rcp.tile([128, 1], F32)
                            nc.vector.reciprocal(rc, po[:, 128:129])
                            nc.vector.tensor_scalar_mul(
                                OT[:, h, qt * 128:(qt + 1) * 128],
                                po[:, 0:128], rc)


            # ---------------- o_proj: out = O^T @ wo_slice ----------------
            with tc.tile_pool(name="otp", bufs=1) as otp, \
                 tc.tile_pool(name="wop", bufs=2) as wop, \
                 tc.tile_pool(name="stgp", bufs=6) as stgp, \
                 tc.tile_pool(name="pout", bufs=6, space="PSUM") as outp:
                # blocked XBAR transposes: OT rows [tok,hd] -> feature-major
                OTT = otp.tile([128, QH, 16, 128], BF16)
                for h in range(QH):
                    nc.sync.dma_start_transpose(OTT[:, h, :, :], OT[:, h, :])
                for n in range(8):
                    nsl = slice(n * 512, (n + 1) * 512)
                    wo_t = wop.tile([128, QH, 512], BF16)
                    nc.scalar.dma_start(wo_t, wo_d.ap()[:, :, nsl])
                    for tt in range(16):
                        psl = slice(tt * 128, (tt + 1) * 128)
                        # split heads 0-3 / 4-7 so the first half can fill
                        # PE gaps while the last chunk's attention finishes
                        psA = outp.tile([128, 512], F32, tag="op")
                        for h in range(4):
                            nc.tensor.matmul(psA, OTT[:, h, tt, :],
                                             wo_t[:, h, :],
                                             start=(h == 0), stop=(h == 3))
                        psB = outp.tile([128, 512], F32, tag="op")
                        for h in range(4, QH):
                            nc.tensor.matmul(psB, OTT[:, h, tt, :],
                                             wo_t[:, h, :],
                                             start=(h == 4),
                                             stop=(h == QH - 1))
                        stg = stgp.tile([128, 512], F32)
                        nc.scalar.copy(stg, psA)
                        nc.vector.tensor_tensor(stg, stg, psB, op=ALU.add)
                        # alternate store queues so the final DMA drain
                        # doesn't serialize on one HWDGE queue
                        q = nc.sync if tt % 2 == 0 else nc.scalar
                        q.dma_start(out_d.ap()[psl, nsl], stg)

    nc.compile()
    return nc


def _host_prep(x, wq, wk, wv, wo):
    bf = ml_dtypes.bfloat16
    scale = HD ** -0.5

    # feature permutation putting rope pairs 16 partitions apart
    perm = np.zeros(128, np.int64)
    for s in range(4):
        for i in range(32):
            perm[32 * s + i] = 16 * s + i if i < 16 else 64 + 16 * s + (i - 16)
    sign = np.array([-1.0 if (i % 32) < 16 else 1.0 for i in range(128)],
                    np.float32)

    inv_freq = 1.0 / (THETA ** (np.arange(0, HD, 2, dtype=np.float32) / HD))
    t = np.arange(T, dtype=np.float32)
    freqs = np.outer(t, inv_freq)                      # [T, 64]
    emb = np.concatenate([freqs, freqs], -1)           # [T, 128]
    cosT = np.ascontiguousarray(np.cos(emb)[:, perm].T).astype(bf)
    sinT = np.ascontiguousarray(
        np.sin(emb)[:, perm].T * sign[:, None]).astype(bf)

    ident = np.eye(128, dtype=np.float32).astype(bf)

    # additive causal masks for the 4 diagonal [128k, 512q] tiles
    kl = np.arange(128)[:, None]
    ql = np.arange(512)[None, :]
    cmask = np.stack(
        [np.where(ql >= d * 128 + kl, 0.0, -1e9).astype(np.float32)
         for d in range(4)], axis=1)                   # [128, 4, 512]
    cmask = np.ascontiguousarray(cmask)

    xT = []
    for b in range(B):
        xT.append(np.ascontiguousarray(x[b].astype(bf).T))

    def wtile(col, permute):
        # [C, 128] -> [8, 128, 512] (cc, part, k*128+f)
        if permute:
            col = col[:, perm]
        r = col.reshape(8, 4, 128, 128).transpose(0, 2, 1, 3)
        return r.reshape(8, 128, 512)

    wqkv, wob = [], []
    for g in range(4):
        tiles = []
        for kvi in range(2):
            tiles.append(wtile(
                wk[:, g * 256 + kvi * 128: g * 256 + (kvi + 1) * 128], True))
        for h in range(8):
            tiles.append(wtile(
                (wq[:, g * 1024 + h * 128: g * 1024 + (h + 1) * 128]
                 * scale), True))
        for kvi in range(2):
            tiles.append(wtile(
                wv[:, g * 256 + kvi * 128: g * 256 + (kvi + 1) * 128], False))
        wqkv.append(np.ascontiguousarray(
            np.stack(tiles, 0).astype(bf)))            # [12, 8, 128, 512]
        wos = wo[g * 1024:(g + 1) * 1024, :]           # [1024, C]
        wob.append(np.ascontiguousarray(
            wos.reshape(QH, 128, C).transpose(1, 0, 2).astype(bf)))

    in_maps = []
    for core in range(NCORES):
        b, g = core // 4, core % 4
        in_maps.append({
            "xT": xT[b], "wqkv": wqkv[g], "wo": wob[g],
            "cosT": cosT, "sinT": sinT,
            "ident": ident, "cmask": cmask,
        })
    return in_maps


def kernel(x, wq, wk, wv, wo, _trace=False, _tmpdir=None):
    if "nc" not in _CACHE:
        _CACHE["nc"] = _build_nc()
    nc = _CACHE["nc"]

    in_maps = _host_prep(x, wq, wk, wv, wo)
    res = run_bass_kernel_spmd(nc, in_maps, core_ids=list(range(NCORES)),
                               trace=_trace, tmpdir=_tmpdir)
    _CACHE["last_results"] = res

    out = np.zeros((B, T, C), np.float32)
    for core in range(NCORES):
        out[core // 4] += res.results[core]["out"]
    return out


# revision 24
# speedup vs baseline: 1.0247x; 1.0247x over previous
"""GQA attention kernel for 8 TRN2 NeuronCores — chunk-pipelined v2.

Problem: B=2, T=2048, C=4096, NH=32 q-heads, NKV=8 kv-heads, HD=128,
RoPE (theta=1e4), causal, f32 I/O.

Sharding: core = (batch b, head-group g): b = core//4, g = core%4.
Each core owns batch b, kv heads {2g, 2g+1} (= q heads 8g..8g+7).

v2 structure (vs phase-serial v1): token chunks of 512 are processed
in a single pipeline — project Q/K/V for chunk qc, then immediately
run causal attention of all 8 heads against K/V chunks <= qc. The
ACT-engine exp of chunk qc then overlaps the PE projections of chunk
qc+1, keeping the PE warm (HAM K=8/8) through the whole attention
phase instead of oscillating. exp is issued in [128, 2, 512] groups
to amortize the ~352-cycle ACTIVATE overhead.

RoPE rotate_half runs on the DVE as a 32-lane quadrant shuffle: q/k
head features are permuted on the host (scores are invariant under a
shared q/k permutation) so each rope pair sits 16 partitions apart
within a quadrant, and the sign folds into the sin table.
"""

import sys

sys.path.insert(0, "/opt/trn_rl_repo")

import numpy as np
import ml_dtypes

import concourse.bass as bass
import concourse.bacc as bacc
import concourse.mybir as mybir
import concourse.tile as tile
from concourse.bass_utils import run_bass_kernel_spmd

BF16 = mybir.dt.bfloat16
F32 = mybir.dt.float32
AF = mybir.ActivationFunctionType
ALU = mybir.AluOpType

B, T, C = 2, 2048, 4096
NH, NKV, HD = 32, 8, 128
THETA = 10000.0
NCORES = 8

QH = 8          # q heads per core
KV = 2          # kv heads per core
QC = 4          # token chunks of 512
CCH = 32        # contraction chunks of 128 over C
NOUT = 12      # projection out tiles per chunk: k0,k1,v0,v1,q0..q7

ROT_MASK = [(i + 16) % 32 for i in range(32)]

_CACHE = {}


def _build_nc():
    nc = bacc.Bacc("TRN2", target_bir_lowering=False, debug=False,
                   enable_asserts=False, num_devices=NCORES)

    xT_d = nc.dram_tensor("xT", [C, T], BF16, kind="ExternalInput")
    wqkv_d = nc.dram_tensor("wqkv", [NOUT, 8, 128, 512], BF16,
                            kind="ExternalInput")
    wo_d = nc.dram_tensor("wo", [128, QH, C], BF16, kind="ExternalInput")
    cos_d = nc.dram_tensor("cosT", [128, T], BF16, kind="ExternalInput")
    sin_d = nc.dram_tensor("sinT", [128, T], BF16, kind="ExternalInput")
    ident_d = nc.dram_tensor("ident", [128, 128], BF16, kind="ExternalInput")
    cmask_d = nc.dram_tensor("cmask", [128, 4, 512], F32, kind="ExternalInput")
    out_d = nc.dram_tensor("out", [T, C], F32, kind="ExternalOutput")

    with tile.TileContext(nc) as tc:
        with tc.tile_pool(name="persist", bufs=1) as pp:
            ident = pp.tile([128, 128], BF16)
            nc.sync.dma_start(ident, ident_d.ap())
            cosT = pp.tile([128, T], BF16)
            sinT = pp.tile([128, T], BF16)
            cmask = pp.tile([128, 4, 512], F32)
            nc.gpsimd.dma_start(cosT, cos_d.ap())
            nc.gpsimd.dma_start(sinT, sin_d.ap())
            nc.gpsimd.dma_start(cmask, cmask_d.ap())

            KTt = pp.tile([128, KV, T], BF16)
            QT = pp.tile([128, 2, QH, 512], BF16)   # double-buffered chunks
            OT = pp.tile([128, QH, T], BF16)
            Vn = pp.tile([128, KV, 16, 132], BF16)
            nc.vector.memset(Vn[:, :, :, 128:129], 1.0)

            X_PIECES0 = (2, 2, 4, 8, 8, 8)
            X_PIECES = (8, 8, 8, 8)

            xview = xT_d.ap().rearrange("(c p) t -> p c t", p=128)

            with tc.tile_pool(name="xp", bufs=2) as xp, \
                 tc.tile_pool(name="wtp", bufs=16) as wtp, \
                 tc.tile_pool(name="vtp", bufs=2) as vtp, \
                 tc.tile_pool(name="rtmp", bufs=2) as rtp, \
                 tc.tile_pool(name="rsp", bufs=2) as rsp, \
                 tc.tile_pool(name="ptp", bufs=10) as ptp, \
                 tc.tile_pool(name="rcp", bufs=4) as rcp, \
                 tc.tile_pool(name="pproj", bufs=2, space="PSUM") as pj, \
                 tc.tile_pool(name="pst", bufs=2, space="PSUM") as stp, \
                 tc.tile_pool(name="ppo", bufs=2, space="PSUM") as pop:

                # HAM warm-up: PE busy while first x/w DMAs land
                for w in range(40):
                    wps = pj.tile([128, 128], BF16, name=f"warm{w}", tag="pj")
                    nc.tensor.transpose(wps, ident, ident)

                def load_x(qc, pieces):
                    tsl = slice(qc * 512, (qc + 1) * 512)
                    xt = xp.tile([128, CCH, 512], BF16, name=f"xt{qc}",
                                 tag="xt")
                    c0 = 0
                    for w in pieces:
                        nc.scalar.dma_start(xt[:, c0:c0 + w, :],
                                            xview[:, c0:c0 + w, tsl])
                        c0 += w
                    return xt

                xts = [None] * QC
                xts[0] = load_x(0, X_PIECES0)

                # init st slots so stale-region exp stays finite
                for i in range(2):
                    sti = stp.tile([128, 2, 512], F32, name=f"sti{i}",
                                   tag="st")
                    nc.vector.memset(sti, 0.0)

                def rope_write(dst, ps, tsl):
                    # dst = ps*cos + quadshuffle(ps)*sin  (sign baked in sin)
                    tmp = rtp.tile([128, 512], F32)
                    nc.vector.stream_shuffle(tmp, ps, ROT_MASK)
                    rs = rsp.tile([128, 512], BF16)
                    nc.vector.tensor_tensor(rs, tmp, sinT[:, tsl],
                                            op=ALU.mult)
                    nc.vector.tensor_tensor(dst, ps, cosT[:, tsl],
                                            op=ALU.mult)
                    nc.vector.tensor_tensor(dst, dst, rs, op=ALU.add)

                for qc in range(QC):
                    tsl = slice(qc * 512, (qc + 1) * 512)
                    qcb = qc % 2

                    # prefetch next chunk's x ahead of this chunk's exps
                    # in the ACT HWDGE queue
                    if qc + 1 < QC:
                        xts[qc + 1] = load_x(qc + 1, X_PIECES)
                    xt = xts[qc]

                    # ---- projections for chunk qc ----
                    for o in range(NOUT):
                        ps = pj.tile([128, 512], F32, name=f"pj{qc}_{o}",
                                     tag="pj")
                        for cc in range(8):
                            wt = wtp.tile([128, 512], BF16)
                            nc.sync.dma_start(wt, wqkv_d.ap()[o, cc])
                            for k in range(4):
                                c = cc * 4 + k
                                nc.tensor.matmul(
                                    ps, wt[:, k * 128:(k + 1) * 128],
                                    xt[:, c, :],
                                    start=(c == 0), stop=(c == CCH - 1))
                        if o < 2:
                            rope_write(KTt[:, o, tsl], ps, tsl)
                        elif o >= 4:
                            h = o - 4
                            rope_write(QT[:, qcb, h, :], ps, tsl)
                        else:
                            kvi = o - 2
                            vt = vtp.tile([128, 512], BF16)
                            nc.vector.tensor_copy(vt, ps)
                            # blocked XBAR transpose: [hd,512] -> 4x[tok,hd]
                            # (contiguous staging: strided transpose targets
                            # are silently mis-written by the XBAR path)
                            vc = vtp.tile([128, 4, 128], BF16, tag="vc")
                            nc.scalar.dma_start_transpose(vc, vt)
                            nc.vector.tensor_copy(
                                Vn[:, kvi, qc * 4:(qc + 1) * 4, 0:128], vc)

                    # ---- attention for chunk qc, all heads ----
                    NG = 2 * qc + 2
                    for h in range(QH):
                        kv = h // 4
                        pts = []
                        for g in range(NG):
                            st = stp.tile([128, 2, 512], F32, tag="st")
                            for i in range(2):
                                kt = 2 * g + i
                                d = kt - 4 * qc
                                ksl = slice(kt * 128, (kt + 1) * 128)
                                if d < 0:
                                    nc.tensor.matmul(
                                        st[:, i, :], KTt[:, kv, ksl],
                                        QT[:, qcb, h, :],
                                        start=True, stop=True)
                                else:
                                    nc.tensor.matmul(
                                        st[:, i, d * 128:],
                                        KTt[:, kv, ksl],
                                        QT[:, qcb, h, d * 128:],
                                        start=True, stop=True)
                            if g >= NG - 2:
                                # only the diagonal-straddling 128-block
                                # needs masking; the below-block region is
                                # stale psum that AV never reads
                                for i in range(2):
                                    d = 2 * g + i - 4 * qc
                                    bsl = slice(d * 128, (d + 1) * 128)
                                    nc.vector.tensor_tensor(
                                        st[:, i, bsl], st[:, i, bsl],
                                        cmask[:, d, bsl], op=ALU.add)
                            ptile = ptp.tile([128, 2, 512], BF16)
                            nc.scalar.activation(ptile, st, AF.Exp)
                            pts.append(ptile)
                        for j in range(4):
                            qt = 4 * qc + j
                            po = pop.tile([128, 129], F32, tag="po")
                            for kt in range(qt + 1):
                                nc.tensor.matmul(
                                    po,
                                    pts[kt // 2][:, kt % 2,
                                                 j * 128:(j + 1) * 128],
                                    Vn[:, kv, kt, 0:129],
                                    start=(kt == 0), stop=(kt == qt))
                            rc = rcp.tile([128, 1], F32)
                            nc.vector.reciprocal(rc, po[:, 128:129])
                            nc.vector.tensor_scalar_mul(
                                OT[:, h, qt * 128:(qt + 1) * 128],
                                po[:, 0:128], rc)


            # ---------------- o_proj: out = O^T @ wo_slice ----------------
            with tc.tile_pool(name="otp", bufs=1) as otp, \
                 tc.tile_pool(name="wop", bufs=2) as wop, \
                 tc.tile_pool(name="stgp", bufs=6) as stgp, \
                 tc.tile_pool(name="pout", bufs=6, space="PSUM") as outp:
                # blocked XBAR transposes: OT rows [tok,hd] -> feature-major
                OTT = otp.tile([128, QH, 16, 128], BF16)
                for h in range(QH):
                    nc.sync.dma_start_transpose(OTT[:, h, :, :], OT[:, h, :])
                for n in range(8):
                    nsl = slice(n * 512, (n + 1) * 512)
                    wo_t = wop.tile([128, QH, 512], BF16)
                    nc.scalar.dma_start(wo_t, wo_d.ap()[:, :, nsl])
                    for tt in range(16):
                        psl = slice(tt * 128, (tt + 1) * 128)
                        # split heads 0-3 / 4-7 so the first half can fill
                        # PE gaps while the last chunk's attention finishes
                        psA = outp.tile([128, 512], F32, tag="op")
                        for h in range(4):
                            nc.tensor.matmul(psA, OTT[:, h, tt, :],
                                             wo_t[:, h, :],
                                             start=(h == 0), stop=(h == 3))
                        psB = outp.tile([128, 512], F32, tag="op")
                        for h in range(4, QH):
                            nc.tensor.matmul(psB, OTT[:, h, tt, :],
                                             wo_t[:, h, :],
                                             start=(h == 4),
                                             stop=(h == QH - 1))
                        stg = stgp.tile([128, 512], F32)
                        nc.scalar.copy(stg, psA)
                        nc.vector.tensor_tensor(stg, stg, psB, op=ALU.add)
                        # alternate store queues so the final DMA drain
                        # doesn't serialize on one HWDGE queue
                        q = nc.sync if tt % 2 == 0 else nc.scalar
                        q.dma_start(out_d.ap()[psl, nsl], stg)

    nc.compile()
    return nc


def _host_prep(x, wq, wk, wv, wo):
    bf = ml_dtypes.bfloat16
    scale = HD ** -0.5

    # feature permutation putting rope pairs 16 partitions apart
    perm = np.zeros(128, np.int64)
    for s in range(4):
        for i in range(32):
            perm[32 * s + i] = 16 * s + i if i < 16 else 64 + 16 * s + (i - 16)
    sign = np.array([-1.0 if (i % 32) < 16 else 1.0 for i in range(128)],
                    np.float32)

    inv_freq = 1.0 / (THETA ** (np.arange(0, HD, 2, dtype=np.float32) / HD))
    t = np.arange(T, dtype=np.float32)
    freqs = np.outer(t, inv_freq)                      # [T, 64]
    emb = np.concatenate([freqs, freqs], -1)           # [T, 128]
    cosT = np.ascontiguousarray(np.cos(emb)[:, perm].T).astype(bf)
    sinT = np.ascontiguousarray(
        np.sin(emb)[:, perm].T * sign[:, None]).astype(bf)

    ident = np.eye(128, dtype=np.float32).astype(bf)

    # additive causal masks for the 4 diagonal [128k, 512q] tiles
    kl = np.arange(128)[:, None]
    ql = np.arange(512)[None, :]
    cmask = np.stack(
        [np.where(ql >= d * 128 + kl, 0.0, -1e9).astype(np.float32)
         for d in range(4)], axis=1)                   # [128, 4, 512]
    cmask = np.ascontiguousarray(cmask)

    xT = []
    for b in range(B):
        xT.append(np.ascontiguousarray(x[b].astype(bf).T))

    def wtile(col, permute):
        # [C, 128] -> [8, 128, 512] (cc, part, k*128+f)
        if permute:
            col = col[:, perm]
        r = col.reshape(8, 4, 128, 128).transpose(0, 2, 1, 3)
        return r.reshape(8, 128, 512)

    wqkv, wob = [], []
    for g in range(4):
        tiles = []
        for kvi in range(2):
            tiles.append(wtile(
                wk[:, g * 256 + kvi * 128: g * 256 + (kvi + 1) * 128], True))
        for kvi in range(2):
            tiles.append(wtile(
                wv[:, g * 256 + kvi * 128: g * 256 + (kvi + 1) * 128], False))
        for h in range(8):
            tiles.append(wtile(
                (wq[:, g * 1024 + h * 128: g * 1024 + (h + 1) * 128]
                 * scale), True))
        wqkv.append(np.ascontiguousarray(
            np.stack(tiles, 0).astype(bf)))            # [12, 8, 128, 512]
        wos = wo[g * 1024:(g + 1) * 1024, :]           # [1024, C]
        wob.append(np.ascontiguousarray(
            wos.reshape(QH, 128, C).transpose(1, 0, 2).astype(bf)))

    in_maps = []
    for core in range(NCORES):
        b, g = core // 4, core % 4
        in_maps.append({
            "xT": xT[b], "wqkv": wqkv[g], "wo": wob[g],
            "cosT": cosT, "sinT": sinT,
            "ident": ident, "cmask": cmask,
        })
    return in_maps


def kernel(x, wq, wk, wv, wo, _trace=False, _tmpdir=None):
    if "nc" not in _CACHE:
        _CACHE["nc"] = _build_nc()
    nc = _CACHE["nc"]

    in_maps = _host_prep(x, wq, wk, wv, wo)
    res = run_bass_kernel_spmd(nc, in_maps, core_ids=list(range(NCORES)),
                               trace=_trace, tmpdir=_tmpdir)
    _CACHE["last_results"] = res

    out = np.zeros((B, T, C), np.float32)
    for core in range(NCORES):
        out[core // 4] += res.results[core]["out"]
    return out


# revision 29
# speedup vs baseline: 1.0271x; 1.0024x over previous
"""GQA attention kernel for 8 TRN2 NeuronCores — chunk-pipelined v2.

Problem: B=2, T=2048, C=4096, NH=32 q-heads, NKV=8 kv-heads, HD=128,
RoPE (theta=1e4), causal, f32 I/O.

Sharding: core = (batch b, head-group g): b = core//4, g = core%4.
Each core owns batch b, kv heads {2g, 2g+1} (= q heads 8g..8g+7).

v2 structure (vs phase-serial v1): token chunks of 512 are processed
in a single pipeline — project Q/K/V for chunk qc, then immediately
run causal attention of all 8 heads against K/V chunks <= qc. The
ACT-engine exp of chunk qc then overlaps the PE projections of chunk
qc+1, keeping the PE warm (HAM K=8/8) through the whole attention
phase instead of oscillating. exp is issued in [128, 2, 512] groups
to amortize the ~352-cycle ACTIVATE overhead.

RoPE rotate_half runs on the DVE as a 32-lane quadrant shuffle: q/k
head features are permuted on the host (scores are invariant under a
shared q/k permutation) so each rope pair sits 16 partitions apart
within a quadrant, and the sign folds into the sin table.
"""

import sys

sys.path.insert(0, "/opt/trn_rl_repo")

import numpy as np
import ml_dtypes

import concourse.bass as bass
import concourse.bacc as bacc
import concourse.mybir as mybir
import concourse.tile as tile
from concourse.bass_utils import run_bass_kernel_spmd

BF16 = mybir.dt.bfloat16
F32 = mybir.dt.float32
AF = mybir.ActivationFunctionType
ALU = mybir.AluOpType

B, T, C = 2, 2048, 4096
NH, NKV, HD = 32, 8, 128
THETA = 10000.0
NCORES = 8

QH = 8          # q heads per core
KV = 2          # kv heads per core
QC = 4          # token chunks of 512
CCH = 32        # contraction chunks of 128 over C
NOUT = 12      # projection out tiles per chunk: k0,k1,v0,v1,q0..q7

ROT_MASK = [(i + 16) % 32 for i in range(32)]

_CACHE = {}


def _build_nc():
    nc = bacc.Bacc("TRN2", target_bir_lowering=False, debug=False,
                   enable_asserts=False, num_devices=NCORES)

    xT_d = nc.dram_tensor("xT", [C, T], BF16, kind="ExternalInput")
    wqkv_d = nc.dram_tensor("wqkv", [NOUT, 8, 128, 512], BF16,
                            kind="ExternalInput")
    wo_d = nc.dram_tensor("wo", [128, QH, C], BF16, kind="ExternalInput")
    cos_d = nc.dram_tensor("cosT", [128, T], BF16, kind="ExternalInput")
    sin_d = nc.dram_tensor("sinT", [128, T], BF16, kind="ExternalInput")
    ident_d = nc.dram_tensor("ident", [128, 128], BF16, kind="ExternalInput")
    cmask_d = nc.dram_tensor("cmask", [128, 4, 512], F32, kind="ExternalInput")
    out_d = nc.dram_tensor("out", [T, C], F32, kind="ExternalOutput")

    with tile.TileContext(nc) as tc:
        with tc.tile_pool(name="persist", bufs=1) as pp:
            ident = pp.tile([128, 128], BF16)
            nc.sync.dma_start(ident, ident_d.ap())
            cosT = pp.tile([128, T], BF16)
            sinT = pp.tile([128, T], BF16)
            cmask = pp.tile([128, 4, 512], F32)
            nc.gpsimd.dma_start(cosT, cos_d.ap())
            nc.gpsimd.dma_start(sinT, sin_d.ap())
            nc.gpsimd.dma_start(cmask, cmask_d.ap())

            KTt = pp.tile([128, KV, T], BF16)
            QT = pp.tile([128, 2, QH, 512], BF16)   # double-buffered chunks
            OT = pp.tile([128, QH, T], BF16)
            Vn = pp.tile([128, KV, 16, 132], BF16)
            nc.vector.memset(Vn[:, :, :, 128:129], 1.0)

            X_PIECES0 = (2, 2, 4, 8, 8, 8)
            X_PIECES = (8, 8, 8, 8)

            xview = xT_d.ap().rearrange("(c p) t -> p c t", p=128)

            with tc.tile_pool(name="xp", bufs=2) as xp, \
                 tc.tile_pool(name="wtp", bufs=16) as wtp, \
                 tc.tile_pool(name="vtp", bufs=2) as vtp, \
                 tc.tile_pool(name="rtmp", bufs=2) as rtp, \
                 tc.tile_pool(name="rsp", bufs=2) as rsp, \
                 tc.tile_pool(name="ptp", bufs=10) as ptp, \
                 tc.tile_pool(name="rcp", bufs=4) as rcp, \
                 tc.tile_pool(name="pproj", bufs=2, space="PSUM") as pj, \
                 tc.tile_pool(name="pst", bufs=2, space="PSUM") as stp, \
                 tc.tile_pool(name="ppo", bufs=2, space="PSUM") as pop:

                # HAM warm-up: PE busy while first x/w DMAs land
                for w in range(56):
                    wps = pj.tile([128, 128], BF16, name=f"warm{w}", tag="pj")
                    nc.tensor.transpose(wps, ident, ident)

                def load_x(qc, pieces):
                    tsl = slice(qc * 512, (qc + 1) * 512)
                    xt = xp.tile([128, CCH, 512], BF16, name=f"xt{qc}",
                                 tag="xt")
                    c0 = 0
                    for w in pieces:
                        nc.scalar.dma_start(xt[:, c0:c0 + w, :],
                                            xview[:, c0:c0 + w, tsl])
                        c0 += w
                    return xt

                xts = [None] * QC
                xts[0] = load_x(0, X_PIECES0)

                # init st slots so stale-region exp stays finite
                for i in range(2):
                    sti = stp.tile([128, 2, 512], F32, name=f"sti{i}",
                                   tag="st")
                    nc.vector.memset(sti, 0.0)

                def rope_write(dst, ps, tsl):
                    # dst = ps*cos + quadshuffle(ps)*sin  (sign baked in sin)
                    tmp = rtp.tile([128, 512], F32)
                    nc.vector.stream_shuffle(tmp, ps, ROT_MASK)
                    rs = rsp.tile([128, 512], BF16)
                    nc.vector.tensor_tensor(rs, tmp, sinT[:, tsl],
                                            op=ALU.mult)
                    nc.vector.tensor_tensor(dst, ps, cosT[:, tsl],
                                            op=ALU.mult)
                    nc.vector.tensor_tensor(dst, dst, rs, op=ALU.add)

                for qc in range(QC):
                    tsl = slice(qc * 512, (qc + 1) * 512)
                    qcb = qc % 2

                    # prefetch next chunk's x ahead of this chunk's exps
                    # in the ACT HWDGE queue
                    if qc + 1 < QC:
                        xts[qc + 1] = load_x(qc + 1, X_PIECES)
                    xt = xts[qc]

                    # ---- projections for chunk qc ----
                    for o in range(NOUT):
                        ps = pj.tile([128, 512], F32, name=f"pj{qc}_{o}",
                                     tag="pj")
                        for cc in range(4):
                            wt = wtp.tile([128, 2, 512], BF16)
                            nc.sync.dma_start(
                                wt, wqkv_d.ap()[o, cc * 2:(cc + 1) * 2]
                                .rearrange("a p f -> p a f"))
                            for k in range(8):
                                c = cc * 8 + k
                                nc.tensor.matmul(
                                    ps,
                                    wt[:, k // 4, (k % 4) * 128:
                                       (k % 4 + 1) * 128],
                                    xt[:, c, :],
                                    start=(c == 0), stop=(c == CCH - 1))
                        if o < 2:
                            rope_write(KTt[:, o, tsl], ps, tsl)
                        elif o >= 4:
                            h = o - 4
                            rope_write(QT[:, qcb, h, :], ps, tsl)
                        else:
                            kvi = o - 2
                            vt = vtp.tile([128, 512], BF16)
                            nc.vector.tensor_copy(vt, ps)
                            # blocked XBAR transpose: [hd,512] -> 4x[tok,hd]
                            # (contiguous staging: strided transpose targets
                            # are silently mis-written by the XBAR path)
                            vc = vtp.tile([128, 4, 128], BF16, tag="vc")
                            nc.scalar.dma_start_transpose(vc, vt)
                            nc.vector.tensor_copy(
                                Vn[:, kvi, qc * 4:(qc + 1) * 4, 0:128], vc)

                    # ---- attention for chunk qc, all heads ----
                    NG = 2 * qc + 2
                    for h in range(QH):
                        kv = h // 4
                        pts = []
                        for g in range(NG):
                            st = stp.tile([128, 2, 512], F32, tag="st")
                            for i in range(2):
                                kt = 2 * g + i
                                d = kt - 4 * qc
                                ksl = slice(kt * 128, (kt + 1) * 128)
                                if d < 0:
                                    nc.tensor.matmul(
                                        st[:, i, :], KTt[:, kv, ksl],
                                        QT[:, qcb, h, :],
                                        start=True, stop=True)
                                else:
                                    nc.tensor.matmul(
                                        st[:, i, d * 128:],
                                        KTt[:, kv, ksl],
                                        QT[:, qcb, h, d * 128:],
                                        start=True, stop=True)
                            if g >= NG - 2:
                                # only the diagonal-straddling 128-block
                                # needs masking; the below-block region is
                                # stale psum that AV never reads
                                for i in range(2):
                                    d = 2 * g + i - 4 * qc
                                    bsl = slice(d * 128, (d + 1) * 128)
                                    nc.vector.tensor_tensor(
                                        st[:, i, bsl], st[:, i, bsl],
                                        cmask[:, d, bsl], op=ALU.add)
                            ptile = ptp.tile([128, 2, 512], BF16)
                            nc.scalar.activation(ptile, st, AF.Exp)
                            pts.append(ptile)
                        for j in range(4):
                            qt = 4 * qc + j
                            po = pop.tile([128, 129], F32, tag="po")
                            for kt in range(qt + 1):
                                nc.tensor.matmul(
                                    po,
                                    pts[kt // 2][:, kt % 2,
                                                 j * 128:(j + 1) * 128],
                                    Vn[:, kv, kt, 0:129],
                                    start=(kt == 0), stop=(kt == qt))
                            rc = rcp.tile([128, 1], F32)
                            nc.vector.reciprocal(rc, po[:, 128:129])
                            nc.vector.tensor_scalar_mul(
                                OT[:, h, qt * 128:(qt + 1) * 128],
                                po[:, 0:128], rc)


            # ---------------- o_proj: out = O^T @ wo_slice ----------------
            with tc.tile_pool(name="otp", bufs=1) as otp, \
                 tc.tile_pool(name="wop", bufs=3) as wop, \
                 tc.tile_pool(name="stgp", bufs=6) as stgp, \
                 tc.tile_pool(name="pout", bufs=6, space="PSUM") as outp:
                # prefetch first wo slices on the (idle) sync queue so
                # o_proj can start while the last chunk's exps drain
                wo_ts = [None] * 8
                for n in range(2):
                    wo_ts[n] = wop.tile([128, QH, 512], BF16, name=f"wo{n}", tag="wo")
                    nc.sync.dma_start(wo_ts[n],
                                      wo_d.ap()[:, :, n * 512:(n + 1) * 512])
                # blocked XBAR transposes: OT rows [tok,hd] -> feature-major
                OTT = otp.tile([128, QH, 16, 128], BF16)
                for h in range(QH):
                    nc.sync.dma_start_transpose(OTT[:, h, :, :], OT[:, h, :])
                for n in range(8):
                    nsl = slice(n * 512, (n + 1) * 512)
                    if n + 2 < 8:
                        wo_ts[n + 2] = wop.tile([128, QH, 512], BF16,
                                                name=f"wo{n+2}", tag="wo")
                        nc.sync.dma_start(
                            wo_ts[n + 2],
                            wo_d.ap()[:, :, (n + 2) * 512:(n + 3) * 512])
                    wo_t = wo_ts[n]
                    for tt in range(16):
                        psl = slice(tt * 128, (tt + 1) * 128)
                        # split heads 0-3 / 4-7 so the first half can fill
                        # PE gaps while the last chunk's attention finishes
                        psA = outp.tile([128, 512], F32, tag="op")
                        for h in range(4):
                            nc.tensor.matmul(psA, OTT[:, h, tt, :],
                                             wo_t[:, h, :],
                                             start=(h == 0), stop=(h == 3))
                        psB = outp.tile([128, 512], F32, tag="op")
                        for h in range(4, QH):
                            nc.tensor.matmul(psB, OTT[:, h, tt, :],
                                             wo_t[:, h, :],
                                             start=(h == 4),
                                             stop=(h == QH - 1))
                        stg = stgp.tile([128, 512], F32)
                        nc.scalar.copy(stg, psA)
                        nc.vector.tensor_tensor(stg, stg, psB, op=ALU.add)
                        # alternate store queues so the final DMA drain
                        # doesn't serialize on one HWDGE queue
                        q = nc.sync if tt % 2 == 0 else nc.scalar
                        q.dma_start(out_d.ap()[psl, nsl], stg)

    nc.compile()
    return nc


def _host_prep(x, wq, wk, wv, wo):
    bf = ml_dtypes.bfloat16
    scale = HD ** -0.5

    # feature permutation putting rope pairs 16 partitions apart
    perm = np.zeros(128, np.int64)
    for s in range(4):
        for i in range(32):
            perm[32 * s + i] = 16 * s + i if i < 16 else 64 + 16 * s + (i - 16)
    sign = np.array([-1.0 if (i % 32) < 16 else 1.0 for i in range(128)],
                    np.float32)

    inv_freq = 1.0 / (THETA ** (np.arange(0, HD, 2, dtype=np.float32) / HD))
    t = np.arange(T, dtype=np.float32)
    freqs = np.outer(t, inv_freq)                      # [T, 64]
    emb = np.concatenate([freqs, freqs], -1)           # [T, 128]
    cosT = np.ascontiguousarray(np.cos(emb)[:, perm].T).astype(bf)
    sinT = np.ascontiguousarray(
        np.sin(emb)[:, perm].T * sign[:, None]).astype(bf)

    ident = np.eye(128, dtype=np.float32).astype(bf)

    # additive causal masks for the 4 diagonal [128k, 512q] tiles
    kl = np.arange(128)[:, None]
    ql = np.arange(512)[None, :]
    cmask = np.stack(
        [np.where(ql >= d * 128 + kl, 0.0, -1e9).astype(np.float32)
         for d in range(4)], axis=1)                   # [128, 4, 512]
    cmask = np.ascontiguousarray(cmask)

    xT = []
    for b in range(B):
        xT.append(np.ascontiguousarray(x[b].astype(bf).T))

    def wtile(col, permute):
        # [C, 128] -> [8, 128, 512] (cc, part, k*128+f)
        if permute:
            col = col[:, perm]
        r = col.reshape(8, 4, 128, 128).transpose(0, 2, 1, 3)
        return r.reshape(8, 128, 512)

    wqkv, wob = [], []
    for g in range(4):
        tiles = []
        for kvi in range(2):
            tiles.append(wtile(
                wk[:, g * 256 + kvi * 128: g * 256 + (kvi + 1) * 128], True))
        for kvi in range(2):
            tiles.append(wtile(
                wv[:, g * 256 + kvi * 128: g * 256 + (kvi + 1) * 128], False))
        for h in range(8):
            tiles.append(wtile(
                (wq[:, g * 1024 + h * 128: g * 1024 + (h + 1) * 128]
                 * scale), True))
        wqkv.append(np.ascontiguousarray(
            np.stack(tiles, 0).astype(bf)))            # [12, 8, 128, 512]
        wos = wo[g * 1024:(g + 1) * 1024, :]           # [1024, C]
        wob.append(np.ascontiguousarray(
            wos.reshape(QH, 128, C).transpose(1, 0, 2).astype(bf)))

    in_maps = []
    for core in range(NCORES):
        b, g = core // 4, core % 4
        in_maps.append({
            "xT": xT[b], "wqkv": wqkv[g], "wo": wob[g],
            "cosT": cosT, "sinT": sinT,
            "ident": ident, "cmask": cmask,
        })
    return in_maps


def kernel(x, wq, wk, wv, wo, _trace=False, _tmpdir=None):
    if "nc" not in _CACHE:
        _CACHE["nc"] = _build_nc()
    nc = _CACHE["nc"]

    in_maps = _host_prep(x, wq, wk, wv, wo)
    res = run_bass_kernel_spmd(nc, in_maps, core_ids=list(range(NCORES)),
                               trace=_trace, tmpdir=_tmpdir)
    _CACHE["last_results"] = res

    out = np.zeros((B, T, C), np.float32)
    for core in range(NCORES):
        out[core // 4] += res.results[core]["out"]
    return out


# revision 32
# speedup vs baseline: 1.0400x; 1.0126x over previous
"""GQA attention kernel for 8 TRN2 NeuronCores — chunk-pipelined v2.

Problem: B=2, T=2048, C=4096, NH=32 q-heads, NKV=8 kv-heads, HD=128,
RoPE (theta=1e4), causal, f32 I/O.

Sharding: core = (batch b, head-group g): b = core//4, g = core%4.
Each core owns batch b, kv heads {2g, 2g+1} (= q heads 8g..8g+7).

v2 structure (vs phase-serial v1): token chunks of 512 are processed
in a single pipeline — project Q/K/V for chunk qc, then immediately
run causal attention of all 8 heads against K/V chunks <= qc. The
ACT-engine exp of chunk qc then overlaps the PE projections of chunk
qc+1, keeping the PE warm (HAM K=8/8) through the whole attention
phase instead of oscillating. exp is issued in [128, 2, 512] groups
to amortize the ~352-cycle ACTIVATE overhead.

RoPE rotate_half runs on the DVE as a 32-lane quadrant shuffle: q/k
head features are permuted on the host (scores are invariant under a
shared q/k permutation) so each rope pair sits 16 partitions apart
within a quadrant, and the sign folds into the sin table.
"""

import sys

sys.path.insert(0, "/opt/trn_rl_repo")

import numpy as np
import ml_dtypes

import concourse.bass as bass
import concourse.bacc as bacc
import concourse.mybir as mybir
import concourse.tile as tile
from concourse.bass_utils import run_bass_kernel_spmd

BF16 = mybir.dt.bfloat16
F32 = mybir.dt.float32
AF = mybir.ActivationFunctionType
ALU = mybir.AluOpType

B, T, C = 2, 2048, 4096
NH, NKV, HD = 32, 8, 128
THETA = 10000.0
NCORES = 8

QH = 8          # q heads per core
KV = 2          # kv heads per core
QC = 4          # token chunks of 512
CCH = 32        # contraction chunks of 128 over C
NOUT = 12      # projection out tiles per chunk: k0,k1,v0,v1,q0..q7

ROT_MASK = [(i + 16) % 32 for i in range(32)]

_CACHE = {}


def _build_nc():
    nc = bacc.Bacc("TRN2", target_bir_lowering=False, debug=False,
                   enable_asserts=False, num_devices=NCORES)

    xT_d = nc.dram_tensor("xT", [C, T], BF16, kind="ExternalInput")
    wqkv_d = nc.dram_tensor("wqkv", [NOUT, 8, 128, 512], BF16,
                            kind="ExternalInput")
    wo_d = nc.dram_tensor("wo", [128, QH, C], BF16, kind="ExternalInput")
    cos_d = nc.dram_tensor("cosT", [128, T], BF16, kind="ExternalInput")
    sin_d = nc.dram_tensor("sinT", [128, T], BF16, kind="ExternalInput")
    ident_d = nc.dram_tensor("ident", [128, 128], BF16, kind="ExternalInput")
    cmask_d = nc.dram_tensor("cmask", [128, 4, 512], F32, kind="ExternalInput")
    out_d = nc.dram_tensor("out", [T, C], F32, kind="ExternalOutput")

    with tile.TileContext(nc) as tc:
        with tc.tile_pool(name="persist", bufs=1) as pp:
            ident = pp.tile([128, 128], BF16)
            nc.sync.dma_start(ident, ident_d.ap())
            cosT = pp.tile([128, T], BF16)
            sinT = pp.tile([128, T], BF16)
            cmask = pp.tile([128, 4, 512], F32)
            nc.gpsimd.dma_start(cosT, cos_d.ap())
            nc.gpsimd.dma_start(sinT, sin_d.ap())
            nc.gpsimd.dma_start(cmask, cmask_d.ap())

            KTt = pp.tile([128, KV, T], BF16)
            QT = pp.tile([128, 2, QH, 512], BF16)   # double-buffered chunks
            OT = pp.tile([128, QH, T], BF16)
            Vn = pp.tile([128, KV, 16, 132], BF16)
            nc.vector.memset(Vn[:, :, :, 128:129], 1.0)

            X_PIECES0 = (2, 2, 4, 8, 8, 8)
            X_PIECES = (8, 8, 8, 8)

            xview = xT_d.ap().rearrange("(c p) t -> p c t", p=128)

            pj = tc.alloc_tile_pool(name="pproj", bufs=2, space="PSUM")
            with tc.tile_pool(name="xp", bufs=2) as xp, \
                 tc.tile_pool(name="wtp", bufs=16) as wtp, \
                 tc.tile_pool(name="vtp", bufs=2) as vtp, \
                 tc.tile_pool(name="rtmp", bufs=2) as rtp, \
                 tc.tile_pool(name="rsp", bufs=2) as rsp, \
                 tc.tile_pool(name="ptp", bufs=10) as ptp, \
                 tc.tile_pool(name="rcp", bufs=4) as rcp, \
                 tc.tile_pool(name="pst", bufs=2, space="PSUM") as stp, \
                 tc.tile_pool(name="ppo", bufs=2, space="PSUM") as pop:

                # HAM warm-up: PE busy while first x/w DMAs land
                for w in range(56):
                    wps = pj.tile([128, 128], BF16, name=f"warm{w}", tag="pj")
                    nc.tensor.transpose(wps, ident, ident)

                def load_x(qc, pieces):
                    tsl = slice(qc * 512, (qc + 1) * 512)
                    xt = xp.tile([128, CCH, 512], BF16, name=f"xt{qc}",
                                 tag="xt")
                    c0 = 0
                    for w in pieces:
                        nc.scalar.dma_start(xt[:, c0:c0 + w, :],
                                            xview[:, c0:c0 + w, tsl])
                        c0 += w
                    return xt

                xts = [None] * QC
                xts[0] = load_x(0, X_PIECES0)

                # init st slots so stale-region exp stays finite
                for i in range(2):
                    sti = stp.tile([128, 2, 512], F32, name=f"sti{i}",
                                   tag="st")
                    nc.vector.memset(sti, 0.0)

                def rope_write(dst, ps, tsl):
                    # dst = ps*cos + quadshuffle(ps)*sin  (sign baked in sin)
                    tmp = rtp.tile([128, 512], F32)
                    nc.vector.stream_shuffle(tmp, ps, ROT_MASK)
                    rs = rsp.tile([128, 512], BF16)
                    nc.vector.tensor_tensor(rs, tmp, sinT[:, tsl],
                                            op=ALU.mult)
                    nc.vector.tensor_tensor(dst, ps, cosT[:, tsl],
                                            op=ALU.mult)
                    nc.vector.tensor_tensor(dst, dst, rs, op=ALU.add)

                for qc in range(QC):
                    tsl = slice(qc * 512, (qc + 1) * 512)
                    qcb = qc % 2

                    # prefetch next chunk's x ahead of this chunk's exps
                    # in the ACT HWDGE queue
                    if qc + 1 < QC:
                        xts[qc + 1] = load_x(qc + 1, X_PIECES)
                    xt = xts[qc]

                    # ---- projections for chunk qc ----
                    for o in range(NOUT):
                        ps = pj.tile([128, 512], F32, name=f"pj{qc}_{o}",
                                     tag="pj")
                        for cc in range(4):
                            wt = wtp.tile([128, 2, 512], BF16)
                            nc.sync.dma_start(
                                wt, wqkv_d.ap()[o, cc * 2:(cc + 1) * 2]
                                .rearrange("a p f -> p a f"))
                            for k in range(8):
                                c = cc * 8 + k
                                nc.tensor.matmul(
                                    ps,
                                    wt[:, k // 4, (k % 4) * 128:
                                       (k % 4 + 1) * 128],
                                    xt[:, c, :],
                                    start=(c == 0), stop=(c == CCH - 1))
                        if o < 2:
                            rope_write(KTt[:, o, tsl], ps, tsl)
                        elif o >= 4:
                            h = o - 4
                            rope_write(QT[:, qcb, h, :], ps, tsl)
                        else:
                            kvi = o - 2
                            vt = vtp.tile([128, 512], BF16)
                            nc.vector.tensor_copy(vt, ps)
                            # blocked XBAR transpose: [hd,512] -> 4x[tok,hd]
                            # (contiguous staging: strided transpose targets
                            # are silently mis-written by the XBAR path)
                            vc = vtp.tile([128, 4, 128], BF16, tag="vc")
                            nc.scalar.dma_start_transpose(vc, vt)
                            nc.vector.tensor_copy(
                                Vn[:, kvi, qc * 4:(qc + 1) * 4, 0:128], vc)

                    # ---- attention for chunk qc, all heads ----
                    NG = 2 * qc + 2
                    for h in range(QH):
                        kv = h // 4
                        pts = []
                        for g in range(NG):
                            st = stp.tile([128, 2, 512], F32, tag="st")
                            for i in range(2):
                                kt = 2 * g + i
                                d = kt - 4 * qc
                                ksl = slice(kt * 128, (kt + 1) * 128)
                                if d < 0:
                                    nc.tensor.matmul(
                                        st[:, i, :], KTt[:, kv, ksl],
                                        QT[:, qcb, h, :],
                                        start=True, stop=True)
                                else:
                                    nc.tensor.matmul(
                                        st[:, i, d * 128:],
                                        KTt[:, kv, ksl],
                                        QT[:, qcb, h, d * 128:],
                                        start=True, stop=True)
                            if g >= NG - 2:
                                # only the diagonal-straddling 128-block
                                # needs masking; the below-block region is
                                # stale psum that AV never reads
                                for i in range(2):
                                    d = 2 * g + i - 4 * qc
                                    bsl = slice(d * 128, (d + 1) * 128)
                                    nc.vector.tensor_tensor(
                                        st[:, i, bsl], st[:, i, bsl],
                                        cmask[:, d, bsl], op=ALU.add)
                            ptile = ptp.tile([128, 2, 512], BF16)
                            nc.scalar.activation(ptile, st, AF.Exp)
                            pts.append(ptile)
                        for j in range(4):
                            qt = 4 * qc + j
                            po = pop.tile([128, 129], F32, tag="po")
                            for kt in range(qt + 1):
                                nc.tensor.matmul(
                                    po,
                                    pts[kt // 2][:, kt % 2,
                                                 j * 128:(j + 1) * 128],
                                    Vn[:, kv, kt, 0:129],
                                    start=(kt == 0), stop=(kt == qt))
                            rc = rcp.tile([128, 1], F32)
                            nc.vector.reciprocal(rc, po[:, 128:129])
                            nc.vector.tensor_scalar_mul(
                                OT[:, h, qt * 128:(qt + 1) * 128],
                                po[:, 0:128], rc)


            pj.release()

            # ---------------- o_proj: out = O^T @ wo_slice ----------------
            with tc.tile_pool(name="otp", bufs=1) as otp, \
                 tc.tile_pool(name="wop", bufs=3) as wop, \
                 tc.tile_pool(name="stgp", bufs=16) as stgp, \
                 tc.tile_pool(name="poutA", bufs=2, space="PSUM") as outpA, \
                 tc.tile_pool(name="poutB", bufs=4, space="PSUM") as outpB:
                # prefetch first wo slices on the (idle) sync queue so
                # o_proj can start while the last chunk's exps drain
                wo_ts = [None] * 8
                for n in range(2):
                    wo_ts[n] = wop.tile([128, QH, 512], BF16, name=f"wo{n}", tag="wo")
                    nc.sync.dma_start(wo_ts[n],
                                      wo_d.ap()[:, :, n * 512:(n + 1) * 512])
                # blocked XBAR transposes: OT rows [tok,hd] -> feature-major
                OTT = otp.tile([128, QH, 16, 128], BF16)
                for h in range(QH):
                    nc.sync.dma_start_transpose(OTT[:, h, :, :], OT[:, h, :])
                for n in range(8):
                    nsl = slice(n * 512, (n + 1) * 512)
                    if n + 2 < 8:
                        wo_ts[n + 2] = wop.tile([128, QH, 512], BF16,
                                                name=f"wo{n+2}", tag="wo")
                        nc.sync.dma_start(
                            wo_ts[n + 2],
                            wo_d.ap()[:, :, (n + 2) * 512:(n + 3) * 512])
                    wo_t = wo_ts[n]
                    for tt in range(16):
                        psl = slice(tt * 128, (tt + 1) * 128)
                        # split heads 0-3 / 4-7 so the first half can fill
                        # PE gaps while the last chunk's attention finishes
                        psA = outpA.tile([128, 512], F32, tag="opA")
                        for h in range(4):
                            nc.tensor.matmul(psA, OTT[:, h, tt, :],
                                             wo_t[:, h, :],
                                             start=(h == 0), stop=(h == 3))
                        psB = outpB.tile([128, 512], F32, tag="opB")
                        for h in range(4, QH):
                            nc.tensor.matmul(psB, OTT[:, h, tt, :],
                                             wo_t[:, h, :],
                                             start=(h == 4),
                                             stop=(h == QH - 1))
                        stg = stgp.tile([128, 512], F32)
                        nc.vector.tensor_copy(stg, psA)
                        nc.vector.tensor_tensor(stg, stg, psB, op=ALU.add)
                        # alternate store queues so the final DMA drain
                        # doesn't serialize on one HWDGE queue
                        q = nc.sync if tt % 2 == 0 else nc.scalar
                        q.dma_start(out_d.ap()[psl, nsl], stg)

    nc.compile()
    return nc


def _host_prep(x, wq, wk, wv, wo):
    bf = ml_dtypes.bfloat16
    scale = HD ** -0.5

    # feature permutation putting rope pairs 16 partitions apart
    perm = np.zeros(128, np.int64)
    for s in range(4):
        for i in range(32):
            perm[32 * s + i] = 16 * s + i if i < 16 else 64 + 16 * s + (i - 16)
    sign = np.array([-1.0 if (i % 32) < 16 else 1.0 for i in range(128)],
                    np.float32)

    inv_freq = 1.0 / (THETA ** (np.arange(0, HD, 2, dtype=np.float32) / HD))
    t = np.arange(T, dtype=np.float32)
    freqs = np.outer(t, inv_freq)                      # [T, 64]
    emb = np.concatenate([freqs, freqs], -1)           # [T, 128]
    cosT = np.ascontiguousarray(np.cos(emb)[:, perm].T).astype(bf)
    sinT = np.ascontiguousarray(
        np.sin(emb)[:, perm].T * sign[:, None]).astype(bf)

    ident = np.eye(128, dtype=np.float32).astype(bf)

    # additive causal masks for the 4 diagonal [128k, 512q] tiles
    kl = np.arange(128)[:, None]
    ql = np.arange(512)[None, :]
    cmask = np.stack(
        [np.where(ql >= d * 128 + kl, 0.0, -1e9).astype(np.float32)
         for d in range(4)], axis=1)                   # [128, 4, 512]
    cmask = np.ascontiguousarray(cmask)

    xT = []
    for b in range(B):
        xT.append(np.ascontiguousarray(x[b].astype(bf).T))

    def wtile(col, permute):
        # [C, 128] -> [8, 128, 512] (cc, part, k*128+f)
        if permute:
            col = col[:, perm]
        r = col.reshape(8, 4, 128, 128).transpose(0, 2, 1, 3)
        return r.reshape(8, 128, 512)

    wqkv, wob = [], []
    for g in range(4):
        tiles = []
        for kvi in range(2):
            tiles.append(wtile(
                wk[:, g * 256 + kvi * 128: g * 256 + (kvi + 1) * 128], True))
        for kvi in range(2):
            tiles.append(wtile(
                wv[:, g * 256 + kvi * 128: g * 256 + (kvi + 1) * 128], False))
        for h in range(8):
            tiles.append(wtile(
                (wq[:, g * 1024 + h * 128: g * 1024 + (h + 1) * 128]
                 * scale), True))
        wqkv.append(np.ascontiguousarray(
            np.stack(tiles, 0).astype(bf)))            # [12, 8, 128, 512]
        wos = wo[g * 1024:(g + 1) * 1024, :]           # [1024, C]
        wob.append(np.ascontiguousarray(
            wos.reshape(QH, 128, C).transpose(1, 0, 2).astype(bf)))

    in_maps = []
    for core in range(NCORES):
        b, g = core // 4, core % 4
        in_maps.append({
            "xT": xT[b], "wqkv": wqkv[g], "wo": wob[g],
            "cosT": cosT, "sinT": sinT,
            "ident": ident, "cmask": cmask,
        })
    return in_maps


def kernel(x, wq, wk, wv, wo, _trace=False, _tmpdir=None):
    if "nc" not in _CACHE:
        _CACHE["nc"] = _build_nc()
    nc = _CACHE["nc"]

    in_maps = _host_prep(x, wq, wk, wv, wo)
    res = run_bass_kernel_spmd(nc, in_maps, core_ids=list(range(NCORES)),
                               trace=_trace, tmpdir=_tmpdir)
    _CACHE["last_results"] = res

    out = np.zeros((B, T, C), np.float32)
    for core in range(NCORES):
        out[core // 4] += res.results[core]["out"]
    return out
